# revision 1
# baseline (speedup 1.0000x reference)
"""Trainium2 Bass kernel: MemoryEfficientAttention block (GroupNorm -> QKV -> 8-head
softmax attention -> out-proj -> LayerNorm -> residual) for hidden_states [4,512,48,48].

Sharding: 8 cores = (batch b = core//2) x (s-half g = core%2). Each core computes
all 8 heads for its 1152 q-rows; k/v over the full 2304 keys. No collectives:
the host permutes hidden-state columns per core so its own q-half comes first,
making the SPMD program core-symmetric. GN is folded into the projections
(per-channel scale into the weights' rows, per-channel shift into a rank-1 bias).
Attention uses scoresT layout [j,i] so the exp output feeds AV directly; softmax
denominators ride along as a ones-column of v.
"""
import sys
import numpy as np

if "/opt/trn_rl_repo" not in sys.path:
    sys.path.insert(0, "/opt/trn_rl_repo")

import ml_dtypes

BF = ml_dtypes.bfloat16

C, S, NH, HD, G = 512, 2304, 8, 64, 32
GPC = C // G          # channels per group = 16
IH = 1152             # local q-rows (s-half)
EPS = 1e-5
NCT = 4               # channel tiles of 128
NDT = 4               # d tiles of 128 (all 8 heads)
NST = 18              # s tiles of 128

SC = [(0, 512), (512, 512), (1024, 512), (1536, 512), (2048, 256)]   # s=2304 chunks
IC = [(0, 512), (512, 512), (1024, 128)]                              # 1152 chunks

_CACHE = {}


def _build():
    import concourse.bass as bass
    import concourse.bacc as bacc
    import concourse.tile as tile
    import concourse.mybir as mybir

    dt = mybir.dt
    F32, F32R, BF16 = dt.float32, dt.float32r, dt.bfloat16
    AF = mybir.ActivationFunctionType
    ALU = mybir.AluOpType

    nc = bacc.Bacc("TRN2", target_bir_lowering=False, debug=False, num_devices=8)

    din = {}
    for name, shape, d in [
        ("hid", [C, S], F32), ("resid", [C, IH], F32),
        ("wq", [C, C], BF16), ("wk", [C, C], BF16), ("wv", [C, C], BF16),
        ("wo", [C, C], BF16),
        ("bq", [128, 4], F32), ("bk", [128, 4], F32), ("bv", [1, C], F32),
        ("bo", [128, 4], F32),
        ("gng", [128, 4], F32), ("gnb", [128, 4], F32),
        ("lng", [128, 4], F32), ("lnb", [128, 4], F32),
        ("ind", [128, 128], F32), ("ones", [128, 128], F32),
        ("masks", [128, 2], F32),
    ]:
        din[name] = nc.dram_tensor(name, shape, d, kind="ExternalInput").ap()
    dout = nc.dram_tensor("out_half", [C, IH], F32, kind="ExternalOutput").ap()

    with tile.TileContext(nc) as tc:
        with (
            tc.tile_pool(name="consts", bufs=1) as cp,
            tc.tile_pool(name="wpool", bufs=1) as wp,
            tc.tile_pool(name="qk", bufs=1) as qkp,
            tc.tile_pool(name="vp", bufs=1) as vp,
            tc.tile_pool(name="ao", bufs=1) as aop,
        ):
            sb = {}
            for name, shape, d in [
                ("bq", [128, 4], F32), ("bk", [128, 4], F32), ("bv", [1, C], F32),
                ("bo", [128, 4], F32), ("gng", [128, 4], F32), ("gnb", [128, 4], F32),
                ("lng", [128, 4], F32), ("lnb", [128, 4], F32),
                ("ind", [128, 128], F32), ("ones", [128, 128], F32),
                ("masks", [128, 2], F32),
            ]:
                if name == "ones":
                    t = cp.tile(shape, F32R, tag=name, name=name)
                    nc.sync.dma_start(t[:], din[name][:].bitcast(F32R))
                else:
                    t = cp.tile(shape, d, tag=name, name=name)
                    nc.sync.dma_start(t[:], din[name][:])
                sb[name] = t
            wq_sb = [wp.tile([128, C], BF16, tag=f"wq{t}", name=f"wq{t}") for t in range(NCT)]
            wk_sb = [wp.tile([128, C], BF16, tag=f"wk{t}", name=f"wk{t}") for t in range(NCT)]
            wv_sb = [wp.tile([128, C], BF16, tag=f"wv{t}", name=f"wv{t}") for t in range(NCT)]
            wo_sb = [wp.tile([128, C], BF16, tag=f"wo{t}", name=f"wo{t}") for t in range(NDT)]
            for t in range(NCT):
                nc.sync.dma_start(wq_sb[t][:], din["wq"][t * 128:(t + 1) * 128, :])
                nc.sync.dma_start(wk_sb[t][:], din["wk"][t * 128:(t + 1) * 128, :])
                nc.sync.dma_start(wv_sb[t][:], din["wv"][t * 128:(t + 1) * 128, :])
                nc.sync.dma_start(wo_sb[t][:], din["wo"][t * 128:(t + 1) * 128, :])

            qTp = [qkp.tile([128, IH], BF16, tag=f"qP{t}", name=f"qP{t}") for t in range(NH)]
            kTb = [qkp.tile([128, S], BF16, tag=f"kT{t}", name=f"kT{t}") for t in range(NDT)]
            VB = NH * 65 + 63  # per-j block, padded so every head has 128 lhsT cols
            v_aug = vp.tile([128, NST * VB], BF16, tag="vaug", name="vaug")
            attn = [aop.tile([128, IH], BF16, tag=f"attn{t}", name=f"attn{t}")
                    for t in range(NDT)]
            oT = [aop.tile([128, IH], F32R, tag=f"oT{t}", name=f"oT{t}")
                  for t in range(NCT)]

            # ================ phase 1: GN stats + projections ================
            with (
                tc.tile_pool(name="hraw", bufs=1) as hp,
                tc.tile_pool(name="hb", bufs=1) as hbp,
                tc.tile_pool(name="p1sb", bufs=2) as p1,
                tc.tile_pool(name="p1ps", bufs=2, space="PSUM") as pp1,
                tc.tile_pool(name="stps", bufs=1, space="PSUM") as stp,
            ):
                hraw = [hp.tile([128, S], F32, tag=f"hraw{t}", name=f"hraw{t}")
                        for t in range(NCT)]
                for t in range(NCT):
                    nc.sync.dma_start(hraw[t][:], din["hid"][t * 128:(t + 1) * 128, :])

                # --- bn_stats per ctile -> per-channel mean/ex2 ---
                m2 = p1.tile([128, 2 * NCT], F32, tag="m2", name="m2")
                for t in range(NCT):
                    st_t = p1.tile([128, 5 * 6], F32, tag="bnst", name="bnst")
                    ag_t = p1.tile([128, 2], F32, tag="bnag", name="bnag")
                    for ci, (c0, cn) in enumerate(SC):
                        nc.vector.bn_stats(st_t[:, ci * 6:(ci + 1) * 6],
                                           hraw[t][:, c0:c0 + cn])
                    nc.vector.bn_aggr(ag_t[:], st_t[:].rearrange("p (n s) -> p n s", s=6))
                    nc.vector.tensor_copy(m2[:, 2 * t:2 * t + 1], ag_t[:, 0:1])
                    nc.vector.scalar_tensor_tensor(
                        m2[:, 2 * t + 1:2 * t + 2], ag_t[:, 0:1], 1.0, ag_t[:, 0:1],
                        op0=ALU.mult, op1=ALU.mult)
                    nc.vector.tensor_add(m2[:, 2 * t + 1:2 * t + 2],
                                         m2[:, 2 * t + 1:2 * t + 2], ag_t[:, 1:2])

                # --- group-average via indicator matmul (replicated) ---
                gst = stp.tile([128, 2 * NCT], F32, tag="gst", name="gst")
                for t in range(NCT):
                    nc.tensor.matmul(gst[:, 2 * t:2 * t + 2], sb["ind"][:],
                                     m2[:, 2 * t:2 * t + 2], start=True, stop=True)

                # --- a/b per channel ---
                mu = p1.tile([128, NCT], F32, tag="mu", name="mu")
                varps = p1.tile([128, NCT], F32, tag="varps", name="varps")
                a_sc = p1.tile([128, NCT], F32, tag="asc", name="asc")
                b_sc = p1.tile([128, NCT], F32, tag="bsc", name="bsc")
                b16 = p1.tile([128, NCT], BF16, tag="b16", name="b16")
                tmp = p1.tile([128, NCT], F32, tag="tmp", name="tmp")
                tmp2 = p1.tile([128, NCT], F32, tag="tmp2", name="tmp2")
                gstv = gst[:].rearrange("p (t k) -> p t k", k=2)
                nc.vector.tensor_copy(mu[:], gstv[:, :, 0])
                nc.vector.tensor_scalar(varps[:], gstv[:, :, 1], 1.0, EPS,
                                        op0=ALU.mult, op1=ALU.add)
                nc.vector.tensor_mul(tmp[:], mu[:], mu[:])
                nc.vector.tensor_sub(varps[:], varps[:], tmp[:])
                nc.scalar.activation(tmp[:], varps[:], AF.Sqrt)
                nc.vector.reciprocal(tmp2[:], tmp[:])
                nc.vector.tensor_mul(tmp[:], tmp2[:], tmp2[:])
                nc.vector.tensor_mul(tmp[:], tmp[:], varps[:])
                nc.vector.tensor_scalar(tmp[:], tmp[:], -0.5, 1.5,
                                        op0=ALU.mult, op1=ALU.add)
                nc.vector.tensor_mul(tmp2[:], tmp2[:], tmp[:])
                nc.vector.tensor_mul(a_sc[:], tmp2[:], sb["gng"][:])
                nc.vector.tensor_mul(tmp[:], mu[:], a_sc[:])
                nc.vector.tensor_sub(b_sc[:], sb["gnb"][:], tmp[:])
                nc.vector.tensor_copy(b16[:], b_sc[:])

                # --- hb16 = hraw * a ---
                hb16 = [hbp.tile([128, S], BF16, tag=f"hb{t}", name=f"hb{t}")
                        for t in range(NCT)]
                for t in range(NCT):
                    nc.vector.tensor_scalar_mul(hb16[t][:], hraw[t][:], a_sc[:, t:t + 1])

                # --- folded bias vectors: b@w + orig bias ---
                bps = stp.tile([128, 8], F32, tag="bps", name="bps")
                for pi, w in enumerate([wq_sb, wk_sb]):
                    for dtt in range(NDT):
                        for t in range(NCT):
                            nc.tensor.matmul(
                                bps[:, pi * 4 + dtt:pi * 4 + dtt + 1],
                                w[t][:, dtt * 128:(dtt + 1) * 128],
                                b16[:, t:t + 1],
                                start=(t == 0), stop=(t == NCT - 1))
                bias_q = p1.tile([128, 4], F32, tag="biasq", name="biasq")
                bias_k = p1.tile([128, 4], F32, tag="biask", name="biask")
                nc.vector.tensor_add(bias_q[:], bps[:, 0:4], sb["bq"][:])
                nc.vector.tensor_add(bias_k[:], bps[:, 4:8], sb["bk"][:])
                bqm = p1.tile([128, 8], F32, tag="bqm", name="bqm")
                for dtt in range(NDT):
                    for hh in range(2):
                        nc.vector.tensor_mul(bqm[:, 2 * dtt + hh:2 * dtt + hh + 1],
                                             bias_q[:, dtt:dtt + 1],
                                             sb["masks"][:, hh:hh + 1])
                bvp = stp.tile([1, C], F32, tag="bvp", name="bvp")
                for t in range(NCT):
                    nc.tensor.matmul(bvp[:], b16[:, t:t + 1], wv_sb[t][:],
                                     start=(t == 0), stop=(t == NCT - 1))
                bvrow = p1.tile([1, C], F32, tag="bvrow", name="bvrow")
                nc.vector.tensor_add(bvrow[:], bvp[:], sb["bv"][:])
                vbias = p1.tile([128, C], F32, tag="vbias", name="vbias")
                nc.gpsimd.partition_broadcast(vbias[:], bvrow[:])

                # --- q projection (local i) + k projection (full s) ---
                for dtt in range(NDT):
                    for (c0, cn) in IC:
                        ps = pp1.tile([128, 512], F32, tag="projps", name="projps")
                        for t in range(NCT):
                            nc.tensor.matmul(
                                ps[:, 0:cn], wq_sb[t][:, dtt * 128:(dtt + 1) * 128],
                                hb16[t][:, c0:c0 + cn],
                                start=(t == 0), stop=(t == NCT - 1))
                        for hh in range(2):
                            nc.vector.tensor_scalar(
                                qTp[2 * dtt + hh][:, c0:c0 + cn], ps[:, 0:cn],
                                sb["masks"][:, hh:hh + 1],
                                bqm[:, 2 * dtt + hh:2 * dtt + hh + 1],
                                op0=ALU.mult, op1=ALU.add)
                for dtt in range(NDT):
                    for (c0, cn) in SC:
                        ps = pp1.tile([128, 512], F32, tag="projps", name="projps")
                        for t in range(NCT):
                            nc.tensor.matmul(
                                ps[:, 0:cn], wk_sb[t][:, dtt * 128:(dtt + 1) * 128],
                                hb16[t][:, c0:c0 + cn],
                                start=(t == 0), stop=(t == NCT - 1))
                        nc.vector.tensor_scalar_add(kTb[dtt][:, c0:c0 + cn],
                                                    ps[:, 0:cn], bias_k[:, dtt:dtt + 1])

                # --- v projection -> v_aug (strided per head, +ones col) ---
                nc.vector.memset(v_aug[:], 1.0)
                for st in range(NST):
                    ps = pp1.tile([128, 512], F32, tag="projps", name="projps")
                    for t in range(NCT):
                        nc.tensor.matmul(
                            ps[:], hb16[t][:, st * 128:(st + 1) * 128],
                            wv_sb[t][:], start=(t == 0), stop=(t == NCT - 1))
                    dst = v_aug[:, st * VB:st * VB + NH * 65].rearrange("p (h k) -> p h k", k=65)
                    nc.vector.tensor_add(
                        dst[:, 0:NH, 0:64],
                        ps[:].rearrange("p (h k) -> p h k", k=64),
                        vbias[:].rearrange("p (h k) -> p h k", k=64))

            # ================ phase 2: attention (8 head-stages) ==============
            with (
                tc.tile_pool(name="ppool", bufs=2) as ppool,
                tc.tile_pool(name="scps", bufs=2, space="PSUM") as scps,
                tc.tile_pool(name="avps", bufs=2, space="PSUM") as avps,
                tc.tile_pool(name="avsb", bufs=3) as avsb,
            ):
                prev = None

                def av_chunk(p_t, h, ci):
                    dtt, ro = h // 2, (h % 2) * 64
                    c0, cn = IC[ci]
                    av = avps.tile([128, 512], F32, tag="av", name="av")
                    for j in range(NST):
                        nc.tensor.matmul(
                            av[:, 0:cn],
                            v_aug[:, j * VB + h * 65:j * VB + h * 65 + 128],
                            p_t[:, j * IH + c0:j * IH + c0 + cn],
                            start=(j == 0), stop=(j == NST - 1))
                    raw = avsb.tile([65, 512], F32, tag="avraw", name="avraw")
                    nc.vector.tensor_copy(raw[:, 0:cn], av[0:65, 0:cn])
                    dr = avsb.tile([1, 512], F32, tag="dr", name="dr")
                    rb = avsb.tile([64, 512], F32, tag="rb", name="rb")
                    nc.vector.reciprocal(dr[0:1, 0:cn], raw[64:65, 0:cn])
                    nc.gpsimd.partition_broadcast(rb[:, 0:cn], dr[0:1, 0:cn])
                    nc.vector.tensor_mul(
                        attn[dtt][ro:ro + 64, c0:c0 + cn],
                        raw[0:64, 0:cn], rb[:, 0:cn])

                for h in range(NH):
                    dtt, ro = h // 2, (h % 2) * 64
                    p_t = ppool.tile([128, NST * IH], BF16, tag="p", name="p")
                    for j in range(NST):
                        sc_t = scps.tile([128, 1536], F32, tag="sc", name="sc")
                        for (c0, cn) in IC:
                            nc.tensor.matmul(
                                sc_t[:, c0:c0 + cn],
                                kTb[dtt][:, j * 128:(j + 1) * 128],
                                qTp[h][:, c0:c0 + cn],
                                start=True, stop=True)
                        nc.scalar.activation(p_t[:, j * IH:(j + 1) * IH],
                                             sc_t[:, 0:IH], AF.Exp, scale=0.125)
                        # interleave AV chunks of the previous head between QK tiles
                        if prev is not None and j in (5, 11, 17):
                            av_chunk(prev, (h - 1), j // 6)
                    prev = p_t
                for ci in range(3):
                    av_chunk(prev, NH - 1, ci)

                # ---- o-proj (bf16, 4-dt accumulation) -> oT sbuf ----
                for cp_i in range(NCT):
                    for (c0, cn) in IC:
                        ps = avps.tile([128, 512], F32, tag="av", name="av")
                        for dtt in range(NDT):
                            nc.tensor.matmul(
                                ps[:, 0:cn],
                                wo_sb[dtt][:, cp_i * 128:(cp_i + 1) * 128],
                                attn[dtt][:, c0:c0 + cn],
                                start=(dtt == 0), stop=(dtt == NDT - 1))
                        nc.vector.tensor_scalar_add(oT[cp_i][:, c0:c0 + cn],
                                                    ps[:, 0:cn],
                                                    sb["bo"][:, cp_i:cp_i + 1])

            # ================ phase 3: LayerNorm + residual ==================
            with (
                tc.tile_pool(name="lnsb", bufs=1) as lp,
                tc.tile_pool(name="lnscr", bufs=2) as lsc,
                tc.tile_pool(name="lnps", bufs=1, space="PSUM") as lps,
            ):
                rsd = [lp.tile([128, IH], F32, tag=f"rsd{t}", name=f"rsd{t}")
                       for t in range(NCT)]
                for t in range(NCT):
                    nc.sync.dma_start(rsd[t][:], din["resid"][t * 128:(t + 1) * 128, :])
                    nc.vector.tensor_scalar_add(rsd[t][:], rsd[t][:],
                                                sb["lnb"][:, t:t + 1])

                psx = lps.tile([128, 1536], F32, tag="psx", name="psx")
                psq = lps.tile([128, 1536], F32, tag="psq", name="psq")
                for t in range(NCT):
                    xsq = lsc.tile([128, IH], F32R, tag="xsq", name="xsq")
                    nc.vector.tensor_mul(xsq[:], oT[t][:], oT[t][:])
                    for (c0, cn) in IC:
                        nc.tensor.matmul(psx[:, c0:c0 + cn], sb["ones"][:],
                                         oT[t][:, c0:c0 + cn],
                                         start=(t == 0), stop=(t == NCT - 1))
                        nc.tensor.matmul(psq[:, c0:c0 + cn], sb["ones"][:],
                                         xsq[:, c0:c0 + cn],
                                         start=(t == 0), stop=(t == NCT - 1))

                mu = lp.tile([128, IH], F32, tag="lnmu", name="lnmu")
                rsq = lp.tile([128, IH], F32, tag="lnrsq", name="lnrsq")
                t1 = lsc.tile([128, IH], F32, tag="lnt1", name="lnt1")
                vps = lsc.tile([128, IH], F32, tag="lnvar", name="lnvar")
                nc.vector.tensor_scalar_mul(mu[:], psx[:, 0:IH], 1.0 / C)
                nc.vector.tensor_scalar(vps[:], psq[:, 0:IH], 1.0 / C, EPS,
                                        op0=ALU.mult, op1=ALU.add)
                nc.vector.tensor_mul(t1[:], mu[:], mu[:])
                nc.vector.tensor_sub(vps[:], vps[:], t1[:])
                nc.scalar.activation(t1[:], vps[:], AF.Sqrt)
                nc.vector.reciprocal(rsq[:], t1[:])
                nc.vector.tensor_mul(t1[:], rsq[:], rsq[:])
                nc.vector.tensor_mul(t1[:], t1[:], vps[:])
                nc.vector.tensor_scalar(t1[:], t1[:], -0.5, 1.5,
                                        op0=ALU.mult, op1=ALU.add)
                nc.vector.tensor_mul(rsq[:], rsq[:], t1[:])   # rsqrt(var+eps)

                for t in range(NCT):
                    ot = lsc.tile([128, IH], F32, tag="lnout", name="lnout")
                    nc.vector.tensor_sub(ot[:], oT[t][:], mu[:])
                    nc.vector.tensor_mul(ot[:], ot[:], rsq[:])
                    nc.vector.scalar_tensor_tensor(
                        ot[:], ot[:], sb["lng"][:, t:t + 1], rsd[t][:],
                        op0=ALU.mult, op1=ALU.add)
                    nc.sync.dma_start(dout[t * 128:(t + 1) * 128, :], ot[:])

    nc.compile()
    return nc


def _prep_inputs(inp):
    hidden = np.ascontiguousarray(np.asarray(inp["hidden_states"], np.float32))
    B = hidden.shape[0]
    wq, wk, wv = (np.asarray(inp[k], np.float32) for k in ("wq", "wk", "wv"))
    wo = np.asarray(inp["wo"], np.float32)
    bq, bk, bv, bo = (np.asarray(inp[k], np.float32) for k in ("bq", "bk", "bv", "bo"))
    gng, gnb = np.asarray(inp["gn_gamma"], np.float32), np.asarray(inp["gn_beta"], np.float32)
    lng, lnb = np.asarray(inp["ln_gamma"], np.float32), np.asarray(inp["ln_beta"], np.float32)

    ind = np.zeros((128, 128), np.float32)
    for c in range(128):
        g0 = (c // GPC) * GPC
        ind[g0:g0 + GPC, c] = 1.0 / GPC
    ones = np.ones((128, 128), np.float32)

    def col4(x):
        return np.ascontiguousarray(x.reshape(4, 128).T)

    wqb, wkb, wvb, wob = (w.astype(BF) for w in (wq, wk, wv, wo))
    consts = {
        "wq": wqb, "wk": wkb, "wv": wvb, "wo": wob,
        "bq": col4(bq), "bk": col4(bk), "bv": np.ascontiguousarray(bv.reshape(1, C)),
        "bo": col4(bo), "gng": col4(gng), "gnb": col4(gnb),
        "lng": col4(lng), "lnb": col4(lnb), "ind": ind, "ones": ones,
        "masks": np.ascontiguousarray(np.stack(
            [np.r_[np.ones(64), np.zeros(64)],
             np.r_[np.zeros(64), np.ones(64)]], axis=1).astype(np.float32)),
    }

    in_maps = []
    for c in range(8):
        b, g = c // 2, c % 2
        hid = hidden[b].reshape(C, S)
        hid_perm = np.ascontiguousarray(np.concatenate(
            [hid[:, g * IH:(g + 1) * IH], hid[:, (1 - g) * IH:(2 - g) * IH]], axis=1))
        m = dict(consts)
        m["hid"] = hid_perm
        m["resid"] = np.ascontiguousarray(hid[:, g * IH:(g + 1) * IH])
        in_maps.append(m)
    return in_maps, B


def kernel(**inp):
    from concourse.bass_utils import run_bass_kernel_spmd

    if "nc" not in _CACHE:
        _CACHE["nc"] = _build()
    nc = _CACHE["nc"]

    in_maps, B = _prep_inputs(inp)
    res = run_bass_kernel_spmd(nc, in_maps, core_ids=list(range(8)))
    outs = [res.results[c]["out_half"] for c in range(8)]
    final = np.zeros((B, C, S), np.float32)
    for b in range(B):
        final[b] = np.concatenate([outs[2 * b], outs[2 * b + 1]], axis=1)
    return final.reshape(B, C, 48, 48)


if __name__ == "__main__":
    _build()
    print("build+compile OK")

